# revision 11
# baseline (speedup 1.0000x reference)
"""MoE (16 routed experts, top-2, + shared expert) on 8 TRN2 NeuronCores.

Strategy (expert-parallel, single fused launch):
  Host: computes the router (fp32 logits/softmax/top-2 — bit-matches the
    jax reference selection), gathers each expert's tokens into dense
    feature-major batches (the all-to-all dispatch), casts everything to
    bf16, and pairs experts onto cores largest-with-smallest so all 8
    cores carry the same padded token load.
  Device (one SPMD launch, all 8 cores): three SwiGLU FFN streams per
    core — the shared expert over its 2048-token data-parallel slice,
    then its two routed experts over their gathered batches.  bf16
    operands (fp32 PSUM accumulation) keep the PE at the 1 cyc/row peak
    while halving DMA vs fp32.  Weight sets are double-buffered so the
    next segment's weights stream in under the current segment's
    matmuls; the block loop is software-pipelined (gate/up of block i+1
    is emitted before down of block i) so the PE never waits on the
    vector engine.
  Host: combine — scales expert outputs by the renormalized top-2
    weights and scatter-adds them with the shared output into the full
    result.

All activations travel transposed (feature-major, token-minor) so every
matmul operand loads with natural DMA strides and zero on-device
transposes.
"""

import numpy as np

# model dims (fixed for this problem)
E, TOPK, C, I = 16, 2, 768, 1536
B, T = 8, 2048
NCORE = 8
NTOK = B * T           # 16384
TPC = NTOK // NCORE    # 2048 tokens per core (shared-expert DP slice)
CK = C // 128          # 6 contraction chunks for C
IK = I // 128          # 12 chunks for I
NBLK = 512             # token block = PE moving-dim per matmul

TRACE = False          # set True (from a driver) to capture NTFF timing
LAST = {}              # timing info from the most recent kernel() call

_progs = {}            # compiled program cache


def _enable_axon_ntff_profiling():
    import sys
    import types

    if "antenv.axon_hooks" not in sys.modules:
        mod = types.ModuleType("antenv.axon_hooks")
        mod._hook = None
        mod.set_axon_ntff_profile_hook = lambda h: setattr(mod, "_hook", h)
        mod.get_axon_ntff_profile_hook = lambda: mod._hook
        sys.modules["antenv.axon_hooks"] = mod
    from antenv.axon_hooks import set_axon_ntff_profile_hook  # type: ignore
    from trn_agent_boot.trn_boot import _ntff_profile_via_ctypes

    set_axon_ntff_profile_hook(_ntff_profile_via_ctypes("/opt/axon/libaxon_pjrt.so"))
    import concourse.bass_utils as bu

    bu.upload_artifacts = lambda tmpdir: f"file://{tmpdir}"


def _blocks(m):
    """Split m tokens into PE-friendly blocks (<=512 each)."""
    out = []
    n0 = 0
    while n0 < m:
        nb = min(NBLK, m - n0)
        out.append((n0, nb))
        n0 += nb
    return out


def _build_fused(cap_a, cap_b):
    """One launch: shared-expert FFN on the DP slice + two routed experts."""
    from contextlib import ExitStack

    import concourse.tile as tile
    from concourse import bacc, mybir

    f32 = mybir.dt.float32
    bf16 = mybir.dt.bfloat16

    nc = bacc.Bacc("TRN2", target_bir_lowering=False, debug=False)
    widths = {"s": TPC, "a": cap_a, "b": cap_b}
    aps = {}
    for s, w in widths.items():
        aps[f"x{s}"] = nc.dram_tensor(f"x{s}", [C, w], bf16, kind="ExternalInput").ap()
        aps[f"wg{s}"] = nc.dram_tensor(f"wg{s}", [C, I], bf16, kind="ExternalInput").ap()
        aps[f"wu{s}"] = nc.dram_tensor(f"wu{s}", [C, I], bf16, kind="ExternalInput").ap()
        aps[f"wd{s}"] = nc.dram_tensor(f"wd{s}", [I, C], bf16, kind="ExternalInput").ap()
        aps[f"y{s}"] = nc.dram_tensor(f"y{s}", [C, w], bf16, kind="ExternalOutput").ap()

    with tile.TileContext(nc) as tc, ExitStack() as ctx:
        wpool = ctx.enter_context(tc.tile_pool(name="weights", bufs=2))
        xpool = ctx.enter_context(tc.tile_pool(name="xp", bufs=3))
        hpool = ctx.enter_context(tc.tile_pool(name="hp", bufs=2))
        gpool = ctx.enter_context(tc.tile_pool(name="gp", bufs=2))
        ypool = ctx.enter_context(tc.tile_pool(name="yp", bufs=4))
        pgu = ctx.enter_context(tc.tile_pool(name="pgu", bufs=2, space="PSUM"))
        pd = ctx.enter_context(tc.tile_pool(name="pd", bufs=2, space="PSUM"))

        def emit_w_gu(s, wg_sb, wu_sb):
            # column-halves, gate before up: the first gate/up chains' weight
            # slices land first so the PE can start ~8us earlier at kernel head
            for lo, hi in ((0, I // 2), (I // 2, I)):
                for ck in range(CK):
                    for w_sb, ap in ((wg_sb, aps[f"wg{s}"]), (wu_sb, aps[f"wu{s}"])):
                        nc.sync.dma_start(
                            out=w_sb[:, ck, lo:hi],
                            in_=ap[ck * 128 : (ck + 1) * 128, lo:hi],
                        )

        def emit_w_d(s, wd_sb):
            for ik in range(IK):
                nc.sync.dma_start(
                    out=wd_sb[:, ik, :], in_=aps[f"wd{s}"][ik * 128 : (ik + 1) * 128, :]
                )

        # segment order: shared first (its inputs are on the critical path),
        # then the routed slots with any partial (non-512) block last so the
        # final copy+DMA tail after the last matmul is as small as possible
        seg_order = ["s"] + sorted(["a", "b"], key=lambda s: widths[s] % NBLK != 0)

        tasks = []
        wtiles = {}
        for s in seg_order:
            wg_sb = wpool.tile([128, CK, I], bf16, tag="wg")
            wu_sb = wpool.tile([128, CK, I], bf16, tag="wu")
            wd_sb = wpool.tile([128, IK, C], bf16, tag="wd")
            wtiles[s] = (wg_sb, wu_sb, wd_sb)
            for n0, nblk in _blocks(widths[s]):
                tasks.append((s, n0, nblk, wg_sb, wu_sb, wd_sb))

        def emit_gate_up(x_sb, wg_sb, wu_sb, h_sb, nblk):
            for ik in range(IK):
                psg = pgu.tile([128, NBLK], f32, tag="psg")
                psu = pgu.tile([128, NBLK], f32, tag="psu")
                for ck in range(CK):
                    nc.tensor.matmul(
                        psg[:, :nblk],
                        lhsT=wg_sb[:, ck, ik * 128 : (ik + 1) * 128],
                        rhs=x_sb[:, ck, :nblk],
                        start=(ck == 0),
                        stop=(ck == CK - 1),
                    )
                for ck in range(CK):
                    nc.tensor.matmul(
                        psu[:, :nblk],
                        lhsT=wu_sb[:, ck, ik * 128 : (ik + 1) * 128],
                        rhs=x_sb[:, ck, :nblk],
                        start=(ck == 0),
                        stop=(ck == CK - 1),
                    )
                ga = gpool.tile([128, NBLK], f32, tag="ga")
                nc.scalar.activation(
                    ga[:, :nblk], psg[:, :nblk], mybir.ActivationFunctionType.Silu
                )
                nc.vector.tensor_mul(h_sb[:, ik, :nblk], ga[:, :nblk], psu[:, :nblk])

        def emit_down(h_sb, wd_sb, y_ap, n0, nblk):
            for ck in range(CK):
                psd = pd.tile([128, NBLK], f32, tag="psd")
                for ik in range(IK):
                    nc.tensor.matmul(
                        psd[:, :nblk],
                        lhsT=wd_sb[:, ik, ck * 128 : (ck + 1) * 128],
                        rhs=h_sb[:, ik, :nblk],
                        start=(ik == 0),
                        stop=(ik == IK - 1),
                    )
                yb = ypool.tile([128, NBLK], bf16, tag="yb")
                nc.vector.tensor_copy(yb[:, :nblk], psd[:, :nblk])
                nc.sync.dma_start(
                    out=y_ap[ck * 128 : (ck + 1) * 128, n0 : n0 + nblk],
                    in_=yb[:, :nblk],
                )

        # software pipeline: gate/up of task i, then down of task i-1, so the
        # PE has matmul work while the vector engine finishes h of task i.
        # Weight DMAs are emitted just-in-time relative to the block stream so
        # the first block's inputs head the DMA queues (emission order is
        # scheduler priority): seg0 gate/up weights + x(0) first, seg0 down
        # weights after block 0, the next segment's weights one block later.
        s0 = seg_order[0]
        s1, s2 = seg_order[1], seg_order[2]
        # the last segment's weight tiles reuse the first segment's buffers
        # (bufs=2), so their dma_start would WAIT for the release — and a
        # waiting issue op head-of-line-blocks every later DMA on the Sync
        # queue.  Emit them only ~2 blocks before segment s2 begins, when the
        # buffer is already free.
        i_s2 = max(2, len(_blocks(widths[s0])) + len(_blocks(widths[s1])) - 2)
        pending = None
        for i, (s, n0, nblk, wg_sb, wu_sb, wd_sb) in enumerate(tasks):
            x_sb = xpool.tile([128, CK, NBLK], bf16, tag="x")
            for ck in range(CK):
                nc.sync.dma_start(
                    out=x_sb[:, ck, :nblk],
                    in_=aps[f"x{s}"][ck * 128 : (ck + 1) * 128, n0 : n0 + nblk],
                )
            if i == 0:
                # x(0) issues first, then the first segment's gate/up weights
                emit_w_gu(s0, wtiles[s0][0], wtiles[s0][1])
            h_sb = hpool.tile([128, IK, NBLK], bf16, tag="h")
            emit_gate_up(x_sb, wg_sb, wu_sb, h_sb, nblk)
            if pending is not None:
                emit_down(*pending)
            pending = (h_sb, wd_sb, aps[f"y{s}"], n0, nblk)
            if i == 0:
                emit_w_d(s0, wtiles[s0][2])
            elif i == 1:
                emit_w_gu(s1, wtiles[s1][0], wtiles[s1][1])
                emit_w_d(s1, wtiles[s1][2])
            if i == i_s2:
                emit_w_gu(s2, wtiles[s2][0], wtiles[s2][1])
                emit_w_d(s2, wtiles[s2][2])
        emit_down(*pending)

    nc.compile()
    return nc


def _run(nc, in_maps, tag):
    from concourse.bass_utils import run_bass_kernel_spmd

    if TRACE:
        _enable_axon_ntff_profiling()
        res = run_bass_kernel_spmd(nc, in_maps, list(range(NCORE)), trace=True)
        LAST[f"{tag}_ns"] = res.exec_time_ns
        if res.instructions_and_trace is not None:
            LAST[f"{tag}_insts"] = res.instructions_and_trace[0]
            LAST[f"{tag}_trace"] = res.instructions_and_trace[1]
    else:
        res = run_bass_kernel_spmd(nc, in_maps, list(range(NCORE)), trace=False)
    return res.results


def kernel(x, w_gate, expert_bias, wg, wu, wd, swg, swu, swd):
    import ml_dtypes

    bf16 = ml_dtypes.bfloat16
    LAST.clear()
    xf = np.asarray(x, np.float32).reshape(NTOK, C)
    w_gate = np.asarray(w_gate, np.float32)
    expert_bias = np.asarray(expert_bias, np.float32)

    # ---- host router (fp32, matches the reference's top-2 selection)
    logits = xf @ w_gate + expert_bias
    m = logits.max(-1, keepdims=True)
    ex = np.exp(logits - m, dtype=np.float32)
    probs = ex / ex.sum(-1, keepdims=True)
    ti = np.argpartition(-probs, TOPK - 1, axis=1)[:, :TOPK]  # unordered top-2
    tp = np.take_along_axis(probs, ti, axis=1)
    tp = tp / tp.sum(-1, keepdims=True)

    # per-expert token index lists + combine weights
    rows = np.repeat(np.arange(NTOK), TOPK)
    exps = ti.ravel()
    wts = tp.ravel().astype(np.float32)
    order = np.argsort(exps, kind="stable")
    rows, exps, wts = rows[order], exps[order], wts[order]
    starts = np.searchsorted(exps, np.arange(E + 1))
    idxs = [rows[starts[e] : starts[e + 1]] for e in range(E)]
    ews = [wts[starts[e] : starts[e + 1]] for e in range(E)]
    counts = np.array([len(ii) for ii in idxs])

    # pair experts onto cores: 8 largest in slot a, 8 smallest in slot b
    rank = np.argsort(-counts, kind="stable")
    slot_a = [int(rank[c]) for c in range(NCORE)]
    slot_b = [int(rank[2 * NCORE - 1 - c]) for c in range(NCORE)]
    cap_a = int(-(-counts[rank[0]] // 128) * 128)
    cap_b = int(-(-counts[rank[NCORE]] // 128) * 128)

    # ---- bf16 staging
    xT = np.ascontiguousarray(xf.T).astype(bf16)  # (C, NTOK) feature-major
    wg_bf = np.asarray(wg, np.float32).astype(bf16)
    wu_bf = np.asarray(wu, np.float32).astype(bf16)
    wd_bf = np.asarray(wd, np.float32).astype(bf16)
    swg_bf = np.asarray(swg, np.float32).astype(bf16)
    swu_bf = np.asarray(swu, np.float32).astype(bf16)
    swd_bf = np.asarray(swd, np.float32).astype(bf16)

    key = (cap_a, cap_b)
    if key not in _progs:
        _progs[key] = _build_fused(cap_a, cap_b)

    in_maps = []
    for c in range(NCORE):
        mcore = {
            "xs": np.ascontiguousarray(xT[:, c * TPC : (c + 1) * TPC]),
            "wgs": swg_bf,
            "wus": swu_bf,
            "wds": swd_bf,
        }
        for s, eidx, cap in (("a", slot_a[c], cap_a), ("b", slot_b[c], cap_b)):
            ii = idxs[eidx]
            xt = np.zeros((C, cap), bf16)
            xt[:, : len(ii)] = xT[:, ii]
            mcore[f"x{s}"] = xt
            mcore[f"wg{s}"] = wg_bf[eidx]
            mcore[f"wu{s}"] = wu_bf[eidx]
            mcore[f"wd{s}"] = wd_bf[eidx]
        in_maps.append(mcore)

    res = _run(_progs[key], in_maps, "fused")

    # ---- host combine: shared + weighted scattered expert outputs
    out = np.empty((NTOK, C), np.float32)
    for c in range(NCORE):
        out[c * TPC : (c + 1) * TPC] = res[c]["ys"].T.astype(np.float32)
    for c in range(NCORE):
        for s, eidx in (("a", slot_a[c]), ("b", slot_b[c])):
            ii = idxs[eidx]
            y = res[c][f"y{s}"][:, : len(ii)].T.astype(np.float32)
            out[ii] += ews[eidx][:, None] * y

    if TRACE:
        LAST["total_ns"] = sum(
            v for k, v in LAST.items() if isinstance(v, int) and k.endswith("_ns")
        )
    return out.reshape(B, T, C)


# revision 14
# speedup vs baseline: 1.0005x; 1.0005x over previous
"""MoE (16 routed experts, top-2, + shared expert) on 8 TRN2 NeuronCores.

Strategy (expert-parallel, single fused launch):
  Host: computes the router (fp32 logits/softmax/top-2 — bit-matches the
    jax reference selection), gathers each expert's tokens into dense
    feature-major batches (the all-to-all dispatch), casts everything to
    bf16, and pairs experts onto cores largest-with-smallest so all 8
    cores carry the same padded token load.
  Device (one SPMD launch, all 8 cores): three SwiGLU FFN streams per
    core — the shared expert over its 2048-token data-parallel slice,
    then its two routed experts over their gathered batches.  bf16
    operands (fp32 PSUM accumulation) keep the PE at the 1 cyc/row peak
    while halving DMA vs fp32.  Weight sets are double-buffered so the
    next segment's weights stream in under the current segment's
    matmuls; the block loop is software-pipelined (gate/up of block i+1
    is emitted before down of block i) so the PE never waits on the
    vector engine.
  Host: combine — scales expert outputs by the renormalized top-2
    weights and scatter-adds them with the shared output into the full
    result.

All activations travel transposed (feature-major, token-minor) so every
matmul operand loads with natural DMA strides and zero on-device
transposes.
"""

import numpy as np

# model dims (fixed for this problem)
E, TOPK, C, I = 16, 2, 768, 1536
B, T = 8, 2048
NCORE = 8
NTOK = B * T           # 16384
TPC = NTOK // NCORE    # 2048 tokens per core (shared-expert DP slice)
CK = C // 128          # 6 contraction chunks for C
IK = I // 128          # 12 chunks for I
NBLK = 512             # token block = PE moving-dim per matmul

TRACE = False          # set True (from a driver) to capture NTFF timing
LAST = {}              # timing info from the most recent kernel() call

_progs = {}            # compiled program cache


def _enable_axon_ntff_profiling():
    import sys
    import types

    if "antenv.axon_hooks" not in sys.modules:
        mod = types.ModuleType("antenv.axon_hooks")
        mod._hook = None
        mod.set_axon_ntff_profile_hook = lambda h: setattr(mod, "_hook", h)
        mod.get_axon_ntff_profile_hook = lambda: mod._hook
        sys.modules["antenv.axon_hooks"] = mod
    from antenv.axon_hooks import set_axon_ntff_profile_hook  # type: ignore
    from trn_agent_boot.trn_boot import _ntff_profile_via_ctypes

    set_axon_ntff_profile_hook(_ntff_profile_via_ctypes("/opt/axon/libaxon_pjrt.so"))
    import concourse.bass_utils as bu

    bu.upload_artifacts = lambda tmpdir: f"file://{tmpdir}"


def _blocks(m):
    """Split m tokens into PE-friendly blocks (<=512 each)."""
    out = []
    n0 = 0
    while n0 < m:
        nb = min(NBLK, m - n0)
        out.append((n0, nb))
        n0 += nb
    return out


def _build_fused(cap_a, cap_b):
    """One launch: shared-expert FFN on the DP slice + two routed experts."""
    from contextlib import ExitStack

    import concourse.tile as tile
    from concourse import bacc, mybir

    f32 = mybir.dt.float32
    bf16 = mybir.dt.bfloat16

    nc = bacc.Bacc("TRN2", target_bir_lowering=False, debug=False)
    widths = {"s": TPC, "a": cap_a, "b": cap_b}
    aps = {}
    for s, w in widths.items():
        aps[f"x{s}"] = nc.dram_tensor(f"x{s}", [C, w], bf16, kind="ExternalInput").ap()
        aps[f"wg{s}"] = nc.dram_tensor(f"wg{s}", [C, I], bf16, kind="ExternalInput").ap()
        aps[f"wu{s}"] = nc.dram_tensor(f"wu{s}", [C, I], bf16, kind="ExternalInput").ap()
        aps[f"wd{s}"] = nc.dram_tensor(f"wd{s}", [I, C], bf16, kind="ExternalInput").ap()
        aps[f"y{s}"] = nc.dram_tensor(f"y{s}", [C, w], bf16, kind="ExternalOutput").ap()

    with tile.TileContext(nc) as tc, ExitStack() as ctx:
        wpool = ctx.enter_context(tc.tile_pool(name="weights", bufs=2))
        xpool = ctx.enter_context(tc.tile_pool(name="xp", bufs=3))
        hpool = ctx.enter_context(tc.tile_pool(name="hp", bufs=2))
        gpool = ctx.enter_context(tc.tile_pool(name="gp", bufs=2))
        ypool = ctx.enter_context(tc.tile_pool(name="yp", bufs=4))
        pgu = ctx.enter_context(tc.tile_pool(name="pgu", bufs=2, space="PSUM"))
        pd = ctx.enter_context(tc.tile_pool(name="pd", bufs=2, space="PSUM"))

        def emit_w_gu(s, wg_sb, wu_sb):
            # column-halves, gate before up: the first gate/up chains' weight
            # slices land first so the PE can start ~8us earlier at kernel head
            for lo, hi in ((0, I // 2), (I // 2, I)):
                for w_sb, ap in ((wg_sb, aps[f"wg{s}"]), (wu_sb, aps[f"wu{s}"])):
                    nc.sync.dma_start(
                        out=w_sb[:, :, lo:hi],
                        in_=ap[:, lo:hi].rearrange("(ck p) i -> p ck i", p=128),
                    )

        def emit_w_d(s, wd_sb):
            nc.sync.dma_start(
                out=wd_sb[:],
                in_=aps[f"wd{s}"].rearrange("(ik p) c -> p ik c", p=128),
            )

        # segment order: shared first (its inputs are on the critical path),
        # then the routed slots with any partial (non-512) block last so the
        # final copy+DMA tail after the last matmul is as small as possible
        seg_order = ["s"] + sorted(["a", "b"], key=lambda s: widths[s] % NBLK != 0)

        tasks = []
        wtiles = {}
        for s in seg_order:
            wg_sb = wpool.tile([128, CK, I], bf16, tag="wg")
            wu_sb = wpool.tile([128, CK, I], bf16, tag="wu")
            wd_sb = wpool.tile([128, IK, C], bf16, tag="wd")
            wtiles[s] = (wg_sb, wu_sb, wd_sb)
            for n0, nblk in _blocks(widths[s]):
                tasks.append((s, n0, nblk, wg_sb, wu_sb, wd_sb))

        def emit_gate_up(x_sb, wg_sb, wu_sb, h_sb, nblk):
            for ik in range(IK):
                psg = pgu.tile([128, NBLK], f32, tag="psg")
                psu = pgu.tile([128, NBLK], f32, tag="psu")
                for ck in range(CK):
                    nc.tensor.matmul(
                        psg[:, :nblk],
                        lhsT=wg_sb[:, ck, ik * 128 : (ik + 1) * 128],
                        rhs=x_sb[:, ck, :nblk],
                        start=(ck == 0),
                        stop=(ck == CK - 1),
                    )
                for ck in range(CK):
                    nc.tensor.matmul(
                        psu[:, :nblk],
                        lhsT=wu_sb[:, ck, ik * 128 : (ik + 1) * 128],
                        rhs=x_sb[:, ck, :nblk],
                        start=(ck == 0),
                        stop=(ck == CK - 1),
                    )
                ga = gpool.tile([128, NBLK], f32, tag="ga")
                nc.scalar.activation(
                    ga[:, :nblk], psg[:, :nblk], mybir.ActivationFunctionType.Silu
                )
                nc.vector.tensor_mul(h_sb[:, ik, :nblk], ga[:, :nblk], psu[:, :nblk])

        def emit_down(h_sb, wd_sb, y_ap, n0, nblk):
            for ck in range(CK):
                psd = pd.tile([128, NBLK], f32, tag="psd")
                for ik in range(IK):
                    nc.tensor.matmul(
                        psd[:, :nblk],
                        lhsT=wd_sb[:, ik, ck * 128 : (ck + 1) * 128],
                        rhs=h_sb[:, ik, :nblk],
                        start=(ik == 0),
                        stop=(ik == IK - 1),
                    )
                yb = ypool.tile([128, NBLK], bf16, tag="yb")
                nc.vector.tensor_copy(yb[:, :nblk], psd[:, :nblk])
                nc.sync.dma_start(
                    out=y_ap[ck * 128 : (ck + 1) * 128, n0 : n0 + nblk],
                    in_=yb[:, :nblk],
                )

        # software pipeline: gate/up of task i, then down of task i-1, so the
        # PE has matmul work while the vector engine finishes h of task i.
        # Weight DMAs are emitted just-in-time relative to the block stream so
        # the first block's inputs head the DMA queues (emission order is
        # scheduler priority): seg0 gate/up weights + x(0) first, seg0 down
        # weights after block 0, the next segment's weights one block later.
        s0 = seg_order[0]
        s1, s2 = seg_order[1], seg_order[2]
        # the last segment's weight tiles reuse the first segment's buffers
        # (bufs=2), so their dma_start would WAIT for the release — and a
        # waiting issue op head-of-line-blocks every later DMA on the Sync
        # queue.  Emit them only ~2 blocks before segment s2 begins, when the
        # buffer is already free.
        i_s2 = max(2, len(_blocks(widths[s0])) + len(_blocks(widths[s1])) - 2)
        pending = None
        for i, (s, n0, nblk, wg_sb, wu_sb, wd_sb) in enumerate(tasks):
            x_sb = xpool.tile([128, CK, NBLK], bf16, tag="x")
            nc.sync.dma_start(
                out=x_sb[:, :, :nblk],
                in_=aps[f"x{s}"][:, n0 : n0 + nblk].rearrange(
                    "(ck p) t -> p ck t", p=128
                ),
            )
            if i == 0:
                # x(0) issues first, then the first segment's gate/up weights
                emit_w_gu(s0, wtiles[s0][0], wtiles[s0][1])
            h_sb = hpool.tile([128, IK, NBLK], bf16, tag="h")
            emit_gate_up(x_sb, wg_sb, wu_sb, h_sb, nblk)
            if pending is not None:
                emit_down(*pending)
            pending = (h_sb, wd_sb, aps[f"y{s}"], n0, nblk)
            if i == 0:
                emit_w_d(s0, wtiles[s0][2])
            elif i == 1:
                emit_w_gu(s1, wtiles[s1][0], wtiles[s1][1])
                emit_w_d(s1, wtiles[s1][2])
            if i == i_s2:
                emit_w_gu(s2, wtiles[s2][0], wtiles[s2][1])
                emit_w_d(s2, wtiles[s2][2])
        emit_down(*pending)

    nc.compile()
    return nc


def _run(nc, in_maps, tag):
    from concourse.bass_utils import run_bass_kernel_spmd

    if TRACE:
        _enable_axon_ntff_profiling()
        res = run_bass_kernel_spmd(nc, in_maps, list(range(NCORE)), trace=True)
        LAST[f"{tag}_ns"] = res.exec_time_ns
        if res.instructions_and_trace is not None:
            LAST[f"{tag}_insts"] = res.instructions_and_trace[0]
            LAST[f"{tag}_trace"] = res.instructions_and_trace[1]
    else:
        res = run_bass_kernel_spmd(nc, in_maps, list(range(NCORE)), trace=False)
    return res.results


def kernel(x, w_gate, expert_bias, wg, wu, wd, swg, swu, swd):
    import ml_dtypes

    bf16 = ml_dtypes.bfloat16
    LAST.clear()
    xf = np.asarray(x, np.float32).reshape(NTOK, C)
    w_gate = np.asarray(w_gate, np.float32)
    expert_bias = np.asarray(expert_bias, np.float32)

    # ---- host router (fp32, matches the reference's top-2 selection)
    logits = xf @ w_gate + expert_bias
    m = logits.max(-1, keepdims=True)
    ex = np.exp(logits - m, dtype=np.float32)
    probs = ex / ex.sum(-1, keepdims=True)
    ti = np.argpartition(-probs, TOPK - 1, axis=1)[:, :TOPK]  # unordered top-2
    tp = np.take_along_axis(probs, ti, axis=1)
    tp = tp / tp.sum(-1, keepdims=True)

    # per-expert token index lists + combine weights
    rows = np.repeat(np.arange(NTOK), TOPK)
    exps = ti.ravel()
    wts = tp.ravel().astype(np.float32)
    order = np.argsort(exps, kind="stable")
    rows, exps, wts = rows[order], exps[order], wts[order]
    starts = np.searchsorted(exps, np.arange(E + 1))
    idxs = [rows[starts[e] : starts[e + 1]] for e in range(E)]
    ews = [wts[starts[e] : starts[e + 1]] for e in range(E)]
    counts = np.array([len(ii) for ii in idxs])

    # pair experts onto cores: 8 largest in slot a, 8 smallest in slot b
    rank = np.argsort(-counts, kind="stable")
    slot_a = [int(rank[c]) for c in range(NCORE)]
    slot_b = [int(rank[2 * NCORE - 1 - c]) for c in range(NCORE)]
    cap_a = int(-(-counts[rank[0]] // 128) * 128)
    cap_b = int(-(-counts[rank[NCORE]] // 128) * 128)

    # ---- bf16 staging
    xT = np.ascontiguousarray(xf.T).astype(bf16)  # (C, NTOK) feature-major
    wg_bf = np.asarray(wg, np.float32).astype(bf16)
    wu_bf = np.asarray(wu, np.float32).astype(bf16)
    wd_bf = np.asarray(wd, np.float32).astype(bf16)
    swg_bf = np.asarray(swg, np.float32).astype(bf16)
    swu_bf = np.asarray(swu, np.float32).astype(bf16)
    swd_bf = np.asarray(swd, np.float32).astype(bf16)

    key = (cap_a, cap_b)
    if key not in _progs:
        _progs[key] = _build_fused(cap_a, cap_b)

    in_maps = []
    for c in range(NCORE):
        mcore = {
            "xs": np.ascontiguousarray(xT[:, c * TPC : (c + 1) * TPC]),
            "wgs": swg_bf,
            "wus": swu_bf,
            "wds": swd_bf,
        }
        for s, eidx, cap in (("a", slot_a[c], cap_a), ("b", slot_b[c], cap_b)):
            ii = idxs[eidx]
            xt = np.zeros((C, cap), bf16)
            xt[:, : len(ii)] = xT[:, ii]
            mcore[f"x{s}"] = xt
            mcore[f"wg{s}"] = wg_bf[eidx]
            mcore[f"wu{s}"] = wu_bf[eidx]
            mcore[f"wd{s}"] = wd_bf[eidx]
        in_maps.append(mcore)

    res = _run(_progs[key], in_maps, "fused")

    # ---- host combine: shared + weighted scattered expert outputs
    out = np.empty((NTOK, C), np.float32)
    for c in range(NCORE):
        out[c * TPC : (c + 1) * TPC] = res[c]["ys"].T.astype(np.float32)
    for c in range(NCORE):
        for s, eidx in (("a", slot_a[c]), ("b", slot_b[c])):
            ii = idxs[eidx]
            y = res[c][f"y{s}"][:, : len(ii)].T.astype(np.float32)
            out[ii] += ews[eidx][:, None] * y

    if TRACE:
        LAST["total_ns"] = sum(
            v for k, v in LAST.items() if isinstance(v, int) and k.endswith("_ns")
        )
    return out.reshape(B, T, C)
